# revision 1
# baseline (speedup 1.0000x reference)
"""LoRA-injected 3x3 conv (MoE-routed adapters), Trainium2 Bass kernel.

Strategy:
 - Host: merge each sample's LoRA adapter into the base conv weight
   (W_eff = conv_w + scale*active * up @ down  -- exact low-rank merge),
   pre-transpose weights to [ci, tap, co] (matmul lhsT layout), zero-pad x
   spatially, shard batch across 8 cores (2 samples each).
 - Device: 3x3 conv as PE matmuls (fp32r: ~1 column/cycle). ci=320 splits
   into partition chunks 128+128+64. The 64-wide tail chunk is packed to
   full K=128 by storing shifted copies of the image in the upper 64
   partitions (shift +2 pairs taps (kh,0)+(kh,2); shift +66 pairs
   (0,1)+(1,1)), so each 512-wide output chunk needs 23 accumulating
   matmuls instead of 27. Weights are the stationary operand and are
   reused across 4 PSUM banks (k-outer, spatial-inner) to amortize
   LDWEIGHTS. Bias-add fused into the PSUM->SBUF copy on ScalarE.
"""

import sys

for _p in ("/opt/trn_rl_repo",):
    if _p not in sys.path:
        sys.path.insert(0, _p)

import numpy as np

B, CIN, COUT, H, W = 16, 320, 320, 64, 64
R, NUM_LORAS, LORA_STRIDE, SCALE = 4, 50, 4, 1.0
NCORES = 8
BLOC = B // NCORES          # samples per core
HP, WP = H + 2, W + 2       # padded spatial
SP = HP * WP                # padded flat spatial per channel
HWFLAT = H * W
HHALF = H // 2              # rows per half-image
HALF_IN = (HHALF + 2) * WP  # 2244 padded elements per half
HALF_BASE = HHALF * WP      # 2112 flat offset of second half
NSL = 4                     # 512-wide spatial chunks per half
SPC = 512
FULL_CHUNKS = [(0, 128), (128, 128)]
CO_CHUNKS = [(0, 128), (128, 128), (256, 64)]
# tap pairs packed into the upper 64 partitions of the ci tail chunk:
# (tap_lo, tap_hi, which shifted tile, kh, kw of tap_lo)
TAIL_TAPS = [
    (0, 2, "a", 0, 0),   # (0,0)+(0,2): shift +2
    (3, 5, "a", 1, 0),   # (1,0)+(1,2)
    (6, 8, "a", 2, 0),   # (2,0)+(2,2)
    (1, 4, "b", 0, 1),   # (0,1)+(1,1): shift +66
    (7, None, "a", 2, 1),  # (2,1) alone, K=64
]

_NC_CACHE = {}


def _build_nc():
    import concourse.bacc as bacc
    import concourse.bass as bass
    import concourse.mybir as mybir
    from concourse import tile

    f32 = mybir.dt.float32
    f32r = mybir.dt.float32r

    nc = bacc.Bacc(None, target_bir_lowering=False)

    xp_d = nc.dram_tensor("xp", [BLOC, CIN, SP], f32r, kind="ExternalInput")
    wt_d = nc.dram_tensor("wt", [BLOC, 256, 9 * COUT], f32r, kind="ExternalInput")
    wp_d = nc.dram_tensor("wp", [BLOC, 128, 5 * COUT], f32r, kind="ExternalInput")
    bias_d = nc.dram_tensor("bias", [128, 3], f32, kind="ExternalInput")
    y_d = nc.dram_tensor("y", [BLOC, COUT, HWFLAT], f32, kind="ExternalOutput")

    with tile.TileContext(nc) as tc:
        with (
            tc.tile_pool(name="io", bufs=2) as io_pool,
            tc.tile_pool(name="const", bufs=1) as cpool,
            tc.tile_pool(name="ostage", bufs=4) as opool,
            tc.tile_pool(name="acc", bufs=8, space=bass.MemorySpace.PSUM) as pspool,
        ):
            bias_t = cpool.tile([128, 3], f32, tag="bias")

            for b in range(BLOC):
                # initial loads spread across idle engine queues so the
                # first matmul group (needs x0+w0 only) gates on ~1.5MB,
                # not the whole sample's working set on one queue
                w01 = []
                for kc, (c0, _) in enumerate(FULL_CHUNKS):
                    wt = io_pool.tile([128, 9 * COUT], f32r, tag=f"w{kc}")
                    eng = nc.gpsimd if kc == 0 else nc.scalar
                    eng.dma_start(out=wt[:], in_=wt_d[b, c0 : c0 + 128, :])
                    w01.append(wt)
                wp = io_pool.tile([128, 5 * COUT], f32r, tag="wp")
                nc.gpsimd.dma_start(out=wp[:], in_=wp_d[b])

                for half in range(2):
                    base = half * HALF_BASE
                    xts = []
                    for kc, (c0, _) in enumerate(FULL_CHUNKS):
                        xt = io_pool.tile([128, HALF_IN], f32r, tag=f"x{kc}")
                        nc.sync.dma_start(
                            out=xt[:], in_=xp_d[b, c0 : c0 + 128, base : base + HALF_IN]
                        )
                        xts.append(xt)
                    # ci tail chunk (64 rows) + shifted copies in partitions 64..127
                    xa = io_pool.tile([128, HALF_IN], f32r, tag="xa")
                    nc.gpsimd.dma_start(out=xa[:64], in_=xp_d[b, 256:320, base : base + HALF_IN])
                    nc.gpsimd.dma_start(
                        out=xa[64:128, 0 : HALF_IN - 2],
                        in_=xp_d[b, 256:320, base + 2 : base + HALF_IN],
                    )
                    xb = io_pool.tile([128, HALF_IN], f32r, tag="xb")
                    nc.sync.dma_start(out=xb[:64], in_=xp_d[b, 256:320, base : base + HALF_IN])
                    nc.sync.dma_start(
                        out=xb[64:128, 0 : HALF_IN - WP],
                        in_=xp_d[b, 256:320, base + WP : base + HALF_IN],
                    )
                    if b == 0 and half == 0:
                        nc.scalar.dma_start(out=bias_t[:], in_=bias_d[:])

                    xvs = [t[:].rearrange("p (h w) -> p h w", w=WP) for t in xts]
                    xav = xa[:].rearrange("p (h w) -> p h w", w=WP)
                    xbv = xb[:].rearrange("p (h w) -> p h w", w=WP)
                    wvs = [t[:].rearrange("p (t c) -> p t c", c=COUT) for t in w01]
                    wpv = wp[:].rearrange("p (q c) -> p q c", c=COUT)

                    for cc, (o0, osz) in enumerate(CO_CHUNKS):
                        # (lhsT, rhs-builder, K) per accumulation step
                        ents = []
                        for kc in range(2):
                            for kh in range(3):
                                for kw in range(3):
                                    ents.append(
                                        (
                                            wvs[kc][:, kh * 3 + kw, o0 : o0 + osz],
                                            (xvs[kc], kh, kw),
                                            128,
                                        )
                                    )
                        for q, (tlo, thi, which, kh, kw) in enumerate(TAIL_TAPS):
                            src = xav if which == "a" else xbv
                            ksz = 128 if thi is not None else 64
                            ents.append(
                                (wpv[:ksz, q, o0 : o0 + osz], (src, kh, kw), ksz)
                            )

                        pss = [
                            pspool.tile([128, SPC], f32, tag="ps", name=f"ps{sl}")
                            for sl in range(NSL)
                        ]
                        last = len(ents) - 1
                        for kj, (lhsT, (src, kh, kw), ksz) in enumerate(ents):
                            for sl in range(NSL):
                                rhs = src[:ksz, 8 * sl + kh : 8 * sl + kh + 8, kw : kw + W]
                                nc.tensor.matmul(
                                    pss[sl][:osz],
                                    lhsT,
                                    rhs,
                                    start=(kj == 0),
                                    stop=(kj == last),
                                )
                        for sl in range(NSL):
                            ob = opool.tile([128, SPC], f32, tag="ob")
                            nc.scalar.activation(
                                ob[:osz],
                                pss[sl][:osz],
                                mybir.ActivationFunctionType.Identity,
                                bias=bias_t[:osz, cc : cc + 1],
                            )
                            o_off = half * (HWFLAT // 2) + sl * SPC
                            nc.sync.dma_start(
                                out=y_d[b, o0 : o0 + osz, o_off : o_off + SPC],
                                in_=ob[:osz],
                            )

    nc.compile()
    return nc


def _get_nc():
    if "nc" not in _NC_CACHE:
        _NC_CACHE["nc"] = _build_nc()
    return _NC_CACHE["nc"]


def _prep_inputs(x, conv_w, conv_b, down_w, up_w, lora_id):
    x = np.asarray(x, dtype=np.float32)
    conv_w = np.asarray(conv_w, dtype=np.float32)
    conv_b = np.asarray(conv_b, dtype=np.float32)
    down_w = np.asarray(down_w, dtype=np.float32)
    up_w = np.asarray(up_w, dtype=np.float32)
    idx = np.asarray(lora_id).astype(np.int64) // LORA_STRIDE
    active = (idx >= 0).astype(np.float32)
    safe = np.clip(idx, 0, NUM_LORAS - 1)

    # Exact LoRA merge: W_lora[b,o,i,kh,kw] = sum_r up[o,r] down[r,i,kh,kw]
    lora = np.matmul(up_w[safe], down_w[safe].reshape(B, R, -1))
    lora = lora.reshape(B, COUT, CIN, 3, 3)
    weff = conv_w[None] + (SCALE * active)[:, None, None, None, None] * lora
    # lhsT layout [b, ci, tap, co]
    wt9 = np.ascontiguousarray(weff.transpose(0, 2, 3, 4, 1)).reshape(B, CIN, 9, COUT)
    wt_main = wt9[:, :256].reshape(B, 256, 9 * COUT)
    # paired tail-chunk weights: [b, 128, 5, co]
    wp_all = np.zeros((B, 128, 5, COUT), dtype=np.float32)
    for q, (tlo, thi, _, _, _) in enumerate(TAIL_TAPS):
        wp_all[:, 0:64, q] = wt9[:, 256:320, tlo]
        if thi is not None:
            wp_all[:, 64:128, q] = wt9[:, 256:320, thi]
    wp_all = wp_all.reshape(B, 128, 5 * COUT)

    xp = np.pad(x, ((0, 0), (0, 0), (1, 1), (1, 1))).reshape(B, CIN, SP)
    bias2 = np.zeros((128, 3), dtype=np.float32)
    for cc, (o0, osz) in enumerate(CO_CHUNKS):
        bias2[:osz, cc] = conv_b[o0 : o0 + osz]

    in_maps = [
        {
            "xp": np.ascontiguousarray(xp[c * BLOC : (c + 1) * BLOC]),
            "wt": np.ascontiguousarray(wt_main[c * BLOC : (c + 1) * BLOC]),
            "wp": np.ascontiguousarray(wp_all[c * BLOC : (c + 1) * BLOC]),
            "bias": bias2,
        }
        for c in range(NCORES)
    ]
    return in_maps


def run_device(in_maps, trace=False, tmpdir=None):
    from concourse.bass_utils import run_bass_kernel_spmd

    nc = _get_nc()
    return run_bass_kernel_spmd(
        nc, in_maps, list(range(NCORES)), trace=trace, tmpdir=tmpdir
    )


def kernel(x, conv_w, conv_b, down_w, up_w, lora_id):
    in_maps = _prep_inputs(x, conv_w, conv_b, down_w, up_w, lora_id)
    out = run_device(in_maps)
    y = np.concatenate([out.results[c]["y"] for c in range(NCORES)], axis=0)
    return np.ascontiguousarray(y.reshape(B, COUT, H, W).astype(np.float32))



# revision 5
# speedup vs baseline: 1.1380x; 1.1380x over previous
"""LoRA-injected 3x3 conv (MoE-routed adapters), Trainium2 Bass kernel.

Strategy:
 - Host: merge each sample's LoRA adapter into the base conv weight
   (W_eff = conv_w + scale*active * up @ down  -- exact low-rank merge),
   pre-transpose weights to [ci, tap, co] (matmul lhsT layout), zero-pad x
   spatially, shard batch across 8 cores (2 samples each).
 - Device: 3x3 conv as PE matmuls (fp32r: ~1 column/cycle). ci=320 splits
   into partition chunks 128+128+64. The 64-wide tail chunk is packed to
   full K=128 by storing shifted copies of the image in the upper 64
   partitions (shift +2 pairs taps (kh,0)+(kh,2); shift +66 pairs
   (0,1)+(1,1)), so each 512-wide output chunk needs 23 accumulating
   matmuls instead of 27. Weights are the stationary operand and are
   reused across 4 PSUM banks (k-outer, spatial-inner) to amortize
   LDWEIGHTS. Bias-add fused into the PSUM->SBUF copy on ScalarE.
"""

import sys

for _p in ("/opt/trn_rl_repo",):
    if _p not in sys.path:
        sys.path.insert(0, _p)

import numpy as np
import ml_dtypes

BF16 = ml_dtypes.bfloat16

B, CIN, COUT, H, W = 16, 320, 320, 64, 64
R, NUM_LORAS, LORA_STRIDE, SCALE = 4, 50, 4, 1.0
NCORES = 8
BLOC = B // NCORES          # samples per core
HP, WP = H + 2, W + 2       # padded spatial
SP = HP * WP                # padded flat spatial per channel
HWFLAT = H * W
HHALF = H // 2              # rows per half-image
HALF_IN = (HHALF + 2) * WP  # 2244 padded elements per half
HALF_BASE = HHALF * WP      # 2112 flat offset of second half
NSL = 4                     # 512-wide spatial chunks per half
SPC = 512
FULL_CHUNKS = [(0, 128), (128, 128)]
CO_CHUNKS = [(0, 128), (128, 128), (256, 64)]
# tap pairs packed into the upper 64 partitions of the ci tail chunk:
# (tap_lo, tap_hi, which shifted tile, kh, kw of tap_lo)
TAIL_TAPS = [
    (0, 2, "a", 0, 0),   # (0,0)+(0,2): shift +2
    (3, 5, "a", 1, 0),   # (1,0)+(1,2)
    (6, 8, "a", 2, 0),   # (2,0)+(2,2)
    (1, 4, "b", 0, 1),   # (0,1)+(1,1): shift +66
    (7, None, "a", 2, 1),  # (2,1) alone, K=64
]

_NC_CACHE = {}


def _build_nc():
    import concourse.bacc as bacc
    import concourse.bass as bass
    import concourse.mybir as mybir
    from concourse import tile

    f32 = mybir.dt.float32
    f32r = mybir.dt.bfloat16  # matmul operand dtype (bf16: 1 cyc/row, 2B, standalone ldweights OK)

    nc = bacc.Bacc(None, target_bir_lowering=False)

    xp_d = nc.dram_tensor("xp", [BLOC, CIN, SP], f32r, kind="ExternalInput")
    wt_d = nc.dram_tensor("wt", [BLOC, 256, 9 * COUT], f32r, kind="ExternalInput")
    wp_d = nc.dram_tensor("wp", [BLOC, 128, 5 * COUT], f32r, kind="ExternalInput")
    bias_d = nc.dram_tensor("bias", [128, 3], f32, kind="ExternalInput")
    y_d = nc.dram_tensor("y", [BLOC, COUT, HWFLAT], f32, kind="ExternalOutput")

    with tile.TileContext(nc) as tc:
        with (
            tc.tile_pool(name="io", bufs=2) as io_pool,
            tc.tile_pool(name="const", bufs=1) as cpool,
            tc.tile_pool(name="ostage", bufs=4) as opool,
            tc.tile_pool(name="acc", bufs=8, space=bass.MemorySpace.PSUM) as pspool,
        ):
            bias_t = cpool.tile([128, 3], f32, tag="bias")

            for b in range(BLOC):
                # initial loads spread across idle engine queues so the
                # first matmul group (needs x0+w0 only) gates on ~1.5MB,
                # not the whole sample's working set on one queue
                w01 = []
                for kc, (c0, _) in enumerate(FULL_CHUNKS):
                    wt = io_pool.tile([128, 9 * COUT], f32r, tag=f"w{kc}")
                    eng = nc.gpsimd if kc == 0 else nc.scalar
                    eng.dma_start(out=wt[:], in_=wt_d[b, c0 : c0 + 128, :])
                    w01.append(wt)
                wp = io_pool.tile([128, 5 * COUT], f32r, tag="wp")
                nc.gpsimd.dma_start(out=wp[:], in_=wp_d[b])

                for half in range(2):
                    base = half * HALF_BASE
                    xts = []
                    for kc, (c0, _) in enumerate(FULL_CHUNKS):
                        xt = io_pool.tile([128, HALF_IN], f32r, tag=f"x{kc}")
                        nc.sync.dma_start(
                            out=xt[:], in_=xp_d[b, c0 : c0 + 128, base : base + HALF_IN]
                        )
                        xts.append(xt)
                    # ci tail chunk (64 rows) + shifted copies in partitions 64..127
                    xa = io_pool.tile([128, HALF_IN], f32r, tag="xa")
                    nc.gpsimd.dma_start(out=xa[:64], in_=xp_d[b, 256:320, base : base + HALF_IN])
                    nc.gpsimd.dma_start(
                        out=xa[64:128, 0 : HALF_IN - 2],
                        in_=xp_d[b, 256:320, base + 2 : base + HALF_IN],
                    )
                    xb = io_pool.tile([128, HALF_IN], f32r, tag="xb")
                    nc.sync.dma_start(out=xb[:64], in_=xp_d[b, 256:320, base : base + HALF_IN])
                    nc.sync.dma_start(
                        out=xb[64:128, 0 : HALF_IN - WP],
                        in_=xp_d[b, 256:320, base + WP : base + HALF_IN],
                    )
                    if b == 0 and half == 0:
                        nc.scalar.dma_start(out=bias_t[:], in_=bias_d[:])

                    xvs = [t[:].rearrange("p (h w) -> p h w", w=WP) for t in xts]
                    xav = xa[:].rearrange("p (h w) -> p h w", w=WP)
                    xbv = xb[:].rearrange("p (h w) -> p h w", w=WP)
                    wvs = [t[:].rearrange("p (t c) -> p t c", c=COUT) for t in w01]
                    wpv = wp[:].rearrange("p (q c) -> p q c", c=COUT)

                    for cc, (o0, osz) in enumerate(CO_CHUNKS):
                        # (lhsT, rhs-builder, K) per accumulation step
                        ents = []
                        for kc in range(2):
                            for kh in range(3):
                                for kw in range(3):
                                    ents.append(
                                        (
                                            wvs[kc][:, kh * 3 + kw, o0 : o0 + osz],
                                            (xvs[kc], kh, kw),
                                            128,
                                        )
                                    )
                        for q, (tlo, thi, which, kh, kw) in enumerate(TAIL_TAPS):
                            src = xav if which == "a" else xbv
                            ksz = 128 if thi is not None else 64
                            ents.append(
                                (wpv[:ksz, q, o0 : o0 + osz], (src, kh, kw), ksz)
                            )

                        pss = [
                            pspool.tile([128, SPC], f32, tag="ps", name=f"ps{sl}")
                            for sl in range(NSL)
                        ]
                        last = len(ents) - 1
                        for kj, (lhsT, (src, kh, kw), ksz) in enumerate(ents):
                            for sl in range(NSL):
                                rhs = src[:ksz, 8 * sl + kh : 8 * sl + kh + 8, kw : kw + W]
                                nc.tensor.matmul(
                                    pss[sl][:osz],
                                    lhsT,
                                    rhs,
                                    start=(kj == 0),
                                    stop=(kj == last),
                                )
                        for sl in range(NSL):
                            ob = opool.tile([128, SPC], f32, tag="ob")
                            nc.scalar.activation(
                                ob[:osz],
                                pss[sl][:osz],
                                mybir.ActivationFunctionType.Identity,
                                bias=bias_t[:osz, cc : cc + 1],
                            )
                            o_off = half * (HWFLAT // 2) + sl * SPC
                            nc.sync.dma_start(
                                out=y_d[b, o0 : o0 + osz, o_off : o_off + SPC],
                                in_=ob[:osz],
                            )

    nc.compile()
    return nc


def _get_nc():
    if "nc" not in _NC_CACHE:
        _NC_CACHE["nc"] = _build_nc()
    return _NC_CACHE["nc"]


def _prep_inputs(x, conv_w, conv_b, down_w, up_w, lora_id):
    x = np.asarray(x, dtype=np.float32)
    conv_w = np.asarray(conv_w, dtype=np.float32)
    conv_b = np.asarray(conv_b, dtype=np.float32)
    down_w = np.asarray(down_w, dtype=np.float32)
    up_w = np.asarray(up_w, dtype=np.float32)
    idx = np.asarray(lora_id).astype(np.int64) // LORA_STRIDE
    active = (idx >= 0).astype(np.float32)
    safe = np.clip(idx, 0, NUM_LORAS - 1)

    # Exact LoRA merge: W_lora[b,o,i,kh,kw] = sum_r up[o,r] down[r,i,kh,kw]
    lora = np.matmul(up_w[safe], down_w[safe].reshape(B, R, -1))
    lora = lora.reshape(B, COUT, CIN, 3, 3)
    weff = conv_w[None] + (SCALE * active)[:, None, None, None, None] * lora
    # lhsT layout [b, ci, tap, co]
    wt9 = np.ascontiguousarray(weff.transpose(0, 2, 3, 4, 1)).reshape(B, CIN, 9, COUT)
    wt_main = wt9[:, :256].reshape(B, 256, 9 * COUT)
    # paired tail-chunk weights: [b, 128, 5, co]
    wp_all = np.zeros((B, 128, 5, COUT), dtype=np.float32)
    for q, (tlo, thi, _, _, _) in enumerate(TAIL_TAPS):
        wp_all[:, 0:64, q] = wt9[:, 256:320, tlo]
        if thi is not None:
            wp_all[:, 64:128, q] = wt9[:, 256:320, thi]
    wp_all = wp_all.reshape(B, 128, 5 * COUT)

    xp = np.pad(x, ((0, 0), (0, 0), (1, 1), (1, 1))).reshape(B, CIN, SP)
    bias2 = np.zeros((128, 3), dtype=np.float32)
    for cc, (o0, osz) in enumerate(CO_CHUNKS):
        bias2[:osz, cc] = conv_b[o0 : o0 + osz]

    xp = xp.astype(BF16)
    wt_main = wt_main.astype(BF16)
    wp_all = wp_all.astype(BF16)
    in_maps = [
        {
            "xp": np.ascontiguousarray(xp[c * BLOC : (c + 1) * BLOC]),
            "wt": np.ascontiguousarray(wt_main[c * BLOC : (c + 1) * BLOC]),
            "wp": np.ascontiguousarray(wp_all[c * BLOC : (c + 1) * BLOC]),
            "bias": bias2,
        }
        for c in range(NCORES)
    ]
    return in_maps


def run_device(in_maps, trace=False, tmpdir=None):
    from concourse.bass_utils import run_bass_kernel_spmd

    nc = _get_nc()
    return run_bass_kernel_spmd(
        nc, in_maps, list(range(NCORES)), trace=trace, tmpdir=tmpdir
    )


def kernel(x, conv_w, conv_b, down_w, up_w, lora_id):
    in_maps = _prep_inputs(x, conv_w, conv_b, down_w, up_w, lora_id)
    out = run_device(in_maps)
    y = np.concatenate([out.results[c]["y"] for c in range(NCORES)], axis=0)
    return np.ascontiguousarray(y.reshape(B, COUT, H, W).astype(np.float32))



# revision 12
# speedup vs baseline: 1.2384x; 1.0883x over previous
"""LoRA-injected 3x3 conv (MoE-routed adapters), Trainium2 Bass kernel.

Winograd F(2x2, 3x3) formulation:
 - Host: merge each sample's LoRA adapter into the base conv weight
   (W_eff = conv_w + scale*active * up @ down -- exact low-rank merge),
   Winograd-transform weights to U = G W G^T (16 taps) and inputs to
   V = B^T d B (16 taps, 32x32=1024 tiles/sample), both bf16. The conv
   becomes 16 independent [ci -> co] matmuls per tile set (2.25x fewer
   PE rows than direct conv). Bias rides as an extra contraction row:
   V row = 1.0, U row = bias on tap (1,1), whose output-transform
   coefficient is 1 for all 4 positions of a tile.
 - Device: per (sample, 512-tile chunk, co chunk): 4 tap-columns x
   4 tap-rows x 3 ci-chunks of accumulating bf16 matmuls into 8 PSUM
   banks; DVE applies output-transform stage 1 (A^T M) reading PSUM
   directly; Pool applies stage 2 ((A^T M) A), writing bf16 y strided;
   host upcasts y to fp32.
 - Batch sharded 2 samples/core across 8 cores.
"""

import sys

for _p in ("/opt/trn_rl_repo",):
    if _p not in sys.path:
        sys.path.insert(0, _p)

import numpy as np
import ml_dtypes

BF16 = ml_dtypes.bfloat16

B, CIN, COUT, H, W = 16, 320, 320, 64, 64
R, NUM_LORAS, LORA_STRIDE, SCALE = 4, 50, 4, 1.0
NCORES = 8
BLOC = B // NCORES            # samples per core
NT = 32 * 32                  # Winograd 2x2 tiles per sample
NTC = 512                     # tiles per device chunk
HWFLAT = H * W
CI_CHUNKS = [(0, 128), (128, 128), (256, 64)]   # ci contraction chunks
CO_CHUNKS = [(0, 128), (128, 128), (256, 64)]
BIAS_TAP = 5                  # tap (1,1): A-coeff 1 for all 4 positions

_NC_CACHE = {}


def _build_nc():
    import concourse.bacc as bacc
    import concourse.bass as bass
    import concourse.mybir as mybir
    from concourse import tile

    f32 = mybir.dt.float32
    bf16 = mybir.dt.bfloat16
    ADD = mybir.AluOpType.add
    SUB = mybir.AluOpType.subtract

    nc = bacc.Bacc(None, target_bir_lowering=False)

    # V: [sample, ci-chunk, 128 rows, 16 taps * 1024 tiles]; ci-chunk 2 rows
    # 64..127 are zero except row 64 = 1.0 on the bias tap.
    v_d = nc.dram_tensor("v", [BLOC, 3, 128, 16 * NT], bf16, kind="ExternalInput")
    # U: [sample, ci-chunk, 128 rows, 16 taps * 320 co]
    u_d = nc.dram_tensor("u", [BLOC, 3, 128, 16 * COUT], bf16, kind="ExternalInput")
    y_d = nc.dram_tensor("y", [BLOC, COUT, HWFLAT], bf16, kind="ExternalOutput")

    with tile.TileContext(nc) as tc:
        with (
            tc.tile_pool(name="vio", bufs=2) as vpool,
            tc.tile_pool(name="uio", bufs=2) as upool,
            tc.tile_pool(name="t1", bufs=2) as tpool,
            tc.tile_pool(name="scr", bufs=1) as spool,
            tc.tile_pool(name="yst", bufs=2) as ypool,
            tc.tile_pool(name="acc", bufs=1, space=bass.MemorySpace.PSUM) as pspool,
        ):
            for b in range(BLOC):
                # U for the whole sample (reused across both tile chunks)
                uts = []
                for ck in range(3):
                    ut = upool.tile([128, 16, COUT], bf16, tag=f"u{ck}")
                    eng = [nc.gpsimd, nc.scalar, nc.gpsimd][ck]
                    eng.dma_start(out=ut[:], in_=u_d[b, ck].rearrange(
                        "p (t c) -> p t c", c=COUT))
                    uts.append(ut)

                for tcix in range(2):
                    t0 = tcix * NTC
                    vts = []
                    for ck in range(3):
                        vt = vpool.tile([128, 16, NTC], bf16, tag=f"v{ck}")
                        eng = [nc.sync, nc.sync, nc.scalar][ck]
                        eng.dma_start(out=vt[:], in_=v_d[b, ck].rearrange(
                            "p (t n) -> p t n", n=NT)[:, :, t0 : t0 + NTC])
                        vts.append(vt)

                    for cc, (o0, osz) in enumerate(CO_CHUNKS):
                        t1 = tpool.tile([128, 8, NTC], f32, tag="t1")
                        yt = ypool.tile([128, 2048], bf16, tag="y")
                        yv = yt[:].rearrange("p (i r j s) -> p i r j s", i=16, r=2, s=2)
                        for c in range(4):
                            pss = []
                            for tr in range(4):
                                tap = 4 * tr + c
                                ps = pspool.tile([128, NTC], f32, tag=f"ps{c % 2}_{tr}")
                                for ck, (k0, kk) in enumerate(CI_CHUNKS):
                                    if ck == 2 and tap == BIAS_TAP:
                                        kk = 65  # extra const-1 row carries bias
                                    nc.tensor.matmul(
                                        ps[:osz],
                                        uts[ck][:kk, tap, o0 : o0 + osz],
                                        vts[ck][:kk, tap, :],
                                        start=(ck == 0),
                                        stop=(ck == 2),
                                    )
                                pss.append(ps)
                            # stage 1 (A^T M) on DVE; only one PSUM operand
                            # per op is legal, so Act stages M1 into SBUF
                            c1 = spool.tile([128, NTC], f32, tag=f"c1{c % 2}")
                            nc.scalar.copy(c1[:osz], pss[1][:osz])
                            nc.vector.tensor_tensor(
                                t1[:osz, 0 + c], pss[0][:osz], c1[:osz], op=ADD)
                            nc.vector.tensor_tensor(
                                t1[:osz, 0 + c], t1[:osz, 0 + c], pss[2][:osz], op=ADD)
                            nc.vector.tensor_tensor(
                                t1[:osz, 4 + c], c1[:osz], pss[2][:osz], op=SUB)
                            nc.vector.tensor_tensor(
                                t1[:osz, 4 + c], t1[:osz, 4 + c], pss[3][:osz], op=SUB)
                        # stage 2 ((A^T M) A) on Pool, bf16 out, strided y write
                        for pr in range(2):
                            tr0 = t1[:osz].rearrange("p q (i j) -> p q i j", j=32)
                            tmp = spool.tile([128, NTC], f32, tag=f"tmp{pr}")
                            tmv = tmp[:osz].rearrange("p (i j) -> p i j", j=32)
                            nc.gpsimd.tensor_tensor(
                                tmp[:osz], t1[:osz, 4 * pr + 0], t1[:osz, 4 * pr + 1],
                                op=ADD)
                            nc.gpsimd.tensor_tensor(
                                yv[:osz, :, pr, :, 0], tmv, tr0[:, 4 * pr + 2], op=ADD)
                            nc.gpsimd.tensor_tensor(
                                tmp[:osz], t1[:osz, 4 * pr + 1], t1[:osz, 4 * pr + 2],
                                op=SUB)
                            nc.gpsimd.tensor_tensor(
                                yv[:osz, :, pr, :, 1], tmv, tr0[:, 4 * pr + 3], op=SUB)
                        nc.scalar.dma_start(
                            out=y_d[b, o0 : o0 + osz, tcix * 2048 : tcix * 2048 + 2048],
                            in_=yt[:osz],
                        )

    nc.compile()
    return nc


def _get_nc():
    if "nc" not in _NC_CACHE:
        _NC_CACHE["nc"] = _build_nc()
    return _NC_CACHE["nc"]


_G = np.array([[1, 0, 0], [0.5, 0.5, 0.5], [0.5, -0.5, 0.5], [0, 0, 1]], np.float32)


def _prep_inputs(x, conv_w, conv_b, down_w, up_w, lora_id):
    x = np.asarray(x, dtype=np.float32)
    conv_w = np.asarray(conv_w, dtype=np.float32)
    conv_b = np.asarray(conv_b, dtype=np.float32)
    down_w = np.asarray(down_w, dtype=np.float32)
    up_w = np.asarray(up_w, dtype=np.float32)
    idx = np.asarray(lora_id).astype(np.int64) // LORA_STRIDE
    active = (idx >= 0).astype(np.float32)
    safe = np.clip(idx, 0, NUM_LORAS - 1)

    # Exact LoRA merge: W_eff[b] = conv_w + scale*active_b * (up_b @ down_b)
    lora = np.matmul(up_w[safe], down_w[safe].reshape(B, R, -1))
    lora = lora.reshape(B, COUT, CIN, 3, 3)
    weff = conv_w[None] + (SCALE * active)[:, None, None, None, None] * lora

    # U[b, a, c, ci, co] = sum_{kh,kw} G[a,kh] G[c,kw] weff[b, co, ci, kh, kw]
    U = np.einsum("ab,cd,xoibd->xacio", _G, _G, weff, optimize=True)
    U = U.reshape(B, 16, CIN, COUT)

    # V via butterflies: xpad [B, ci, 66, 66]
    xp = np.pad(x, ((0, 0), (0, 0), (1, 1), (1, 1)))
    r = xp
    t = np.stack([
        r[:, :, 0:64:2] - r[:, :, 2:66:2],
        r[:, :, 1:65:2] + r[:, :, 2:66:2],
        r[:, :, 2:66:2] - r[:, :, 1:65:2],
        r[:, :, 1:65:2] - r[:, :, 3:67:2],
    ], axis=1)  # [B, 4q, ci, 32, 66]
    V = np.stack([
        t[..., 0:64:2] - t[..., 2:66:2],
        t[..., 1:65:2] + t[..., 2:66:2],
        t[..., 2:66:2] - t[..., 1:65:2],
        t[..., 1:65:2] - t[..., 3:67:2],
    ], axis=2)  # [B, 4q, 4p, ci, 32, 32]
    V = V.reshape(B, 16, CIN, NT)

    # chunked DRAM layouts with bias row on tap (1,1), ci-chunk 2, row 64
    v_all = np.zeros((B, 3, 128, 16, NT), dtype=BF16)
    u_all = np.zeros((B, 3, 128, 16, COUT), dtype=BF16)
    for ck, (k0, kk) in enumerate(CI_CHUNKS):
        v_all[:, ck, :kk] = V[:, :, k0 : k0 + kk].transpose(0, 2, 1, 3).astype(BF16)
        u_all[:, ck, :kk] = U[:, :, k0 : k0 + kk].transpose(0, 2, 1, 3).astype(BF16)
    v_all[:, 2, 64, BIAS_TAP, :] = np.float32(1.0).astype(BF16)
    u_all[:, 2, 64, BIAS_TAP, :] = conv_b.astype(BF16)[None, :]

    v_all = v_all.reshape(B, 3, 128, 16 * NT)
    u_all = u_all.reshape(B, 3, 128, 16 * COUT)

    in_maps = [
        {
            "v": np.ascontiguousarray(v_all[c * BLOC : (c + 1) * BLOC]),
            "u": np.ascontiguousarray(u_all[c * BLOC : (c + 1) * BLOC]),
        }
        for c in range(NCORES)
    ]
    return in_maps


def run_device(in_maps, trace=False, tmpdir=None):
    from concourse.bass_utils import run_bass_kernel_spmd

    nc = _get_nc()
    return run_bass_kernel_spmd(
        nc, in_maps, list(range(NCORES)), trace=trace, tmpdir=tmpdir
    )


def kernel(x, conv_w, conv_b, down_w, up_w, lora_id):
    in_maps = _prep_inputs(x, conv_w, conv_b, down_w, up_w, lora_id)
    out = run_device(in_maps)
    y = np.concatenate([out.results[c]["y"] for c in range(NCORES)], axis=0)
    y = y.astype(np.float32).reshape(B, COUT, H, W)
    return np.ascontiguousarray(y)
